# revision 19
# baseline (speedup 1.0000x reference)
"""Cache-offloaded transformer decode step on 8 TRN2 NeuronCores, v3.

vs v2: (1) cross-core entry barrier before any real work so the measured
window starts aligned (PJRT dispatches the 8 per-core executions staggered;
without the barrier core 0 idles ~1.8ms inside its measured span waiting for
the last core's first all-reduce contribution); (2) every large load (qkv /
ffn1 / ffn2 / K / V / vocab head) split into ~1-2MB chunks, double- or
triple-buffered, all issued on the sync HWDGE queue in consumption order and
emitted right after the compute that frees the slot — so the weight stream
for layer l+1 overlaps layer l's compute instead of serializing behind it;
(3) the 16 vocab-head chunks stream during layer 7 + the head matmuls.
"""

import math
import os
import sys

import numpy as np

for _p in ("/opt/trn_rl_repo",):
    if _p not in sys.path:
        sys.path.append(_p)

import concourse.bass as bass
import concourse.bacc as bacc
import concourse.mybir as mybir
import concourse.tile as tile
from concourse.bass_utils import run_bass_kernel_spmd

F32 = mybir.dt.float32
BF16 = mybir.dt.bfloat16
AF = mybir.ActivationFunctionType
AX = mybir.AxisListType

NCORES = 8
B, S, HID, NH, L, V = 2, 2048, 2048, 16, 8, 32000
D = HID // NH
OFF = (S + 1) // 2                # 1024
R = S - OFF + 1                   # 1025
HPC = NH // NCORES                # 2
PAIRS = B * HPC                   # 4
QKV_PC = 3 * D * HPC              # 768
FFN_PC = 4 * HID // NCORES        # 1024
VPC = V // NCORES                 # 4000
VPCP = 4096
NC16 = HID // 128                 # 16
KCH = 17
KW = 2049                         # keys per pair
VW = KCH * 128                    # 2176 v cols per pair
INVSQ = 1.0 / math.sqrt(D)

_cached = None


def _ln(nc, sb, ps, XT, gT, bT, ones, onesr, eps, name):
    """LayerNorm of XT -> (xn_f32, xn_bf16) tiles [128, 32]."""
    sq = sb.tile([128, 32], F32, tag="ln_sq", name=f"sq_{name}")
    nc.vector.tensor_mul(sq[:], XT[:], XT[:])
    st = ps.tile([1, 64], F32, tag="ps_small", name=f"st_{name}")
    nc.tensor.matmul(st[0:1, 0:32], ones[:], XT[:], start=True, stop=True)
    nc.tensor.matmul(st[0:1, 32:64], ones[:], sq[:], start=True, stop=True)
    red = sb.tile([1, 4], F32, tag="ln_red", name=f"red_{name}")
    nc.vector.reduce_sum(
        red[:], st[0:1, :].rearrange("p (t b c) -> p (t b) c", t=2, b=2), axis=AX.X
    )
    mr = sb.tile([1, 4], F32, tag="ln_mr", name=f"mr_{name}")
    nc.vector.tensor_scalar_mul(mr[0:1, :], red[0:1, :], 1.0 / HID)
    var = sb.tile([1, 4], F32, tag="ln_var", name=f"var_{name}")
    nc.vector.tensor_mul(var[0:1, 0:2], mr[0:1, 0:2], mr[0:1, 0:2])
    nc.vector.tensor_sub(var[0:1, 0:2], mr[0:1, 2:4], var[0:1, 0:2])
    nc.scalar.activation(var[0:1, 2:4], var[0:1, 0:2], AF.Sqrt,
                         bias=eps[0:1, 0:1])
    nc.vector.reciprocal(mr[0:1, 2:4], var[0:1, 2:4])
    mrb = ps.tile([128, 4], F32, tag="ps_small", name=f"mrb_{name}")
    nc.tensor.matmul(mrb[:], onesr[0:1, :], mr[0:1, :], start=True, stop=True)

    xn = sb.tile([128, 32], F32, tag="xn", name=f"xn_{name}")
    xv = xn[:, :].rearrange("p (b c) -> p b c", b=2)
    XTv = XT[:, :].rearrange("p (b c) -> p b c", b=2)
    m_b = mrb[:, 0:2].unsqueeze(2).broadcast_to([128, 2, 16])
    r_b = mrb[:, 2:4].unsqueeze(2).broadcast_to([128, 2, 16])
    g_b = gT[:, :].unsqueeze(1).broadcast_to([128, 2, 16])
    b_b = bT[:, :].unsqueeze(1).broadcast_to([128, 2, 16])
    nc.vector.tensor_sub(xv, XTv, m_b)
    nc.vector.tensor_mul(xv, xv, r_b)
    nc.vector.tensor_mul(xv, xv, g_b)
    xnb = sb.tile([128, 32], BF16, tag="xnb", name=f"xnb_{name}")
    xnbv = xnb[:, :].rearrange("p (b c) -> p b c", b=2)
    nc.vector.tensor_add(xnbv, xv, b_b)
    return xn, xnb


def _build():
    nc = bacc.Bacc("TRN2", target_bir_lowering=False, debug=False,
                   num_devices=NCORES)

    xpet = nc.dram_tensor("xpet", [128, 32], F32, kind="ExternalInput").ap()
    lng = nc.dram_tensor("lng_t", [128, 16], F32, kind="ExternalInput").ap()
    lnb = nc.dram_tensor("lnb_t", [128, 16], F32, kind="ExternalInput").ap()
    ident_in = nc.dram_tensor("ident", [128, 128], BF16, kind="ExternalInput").ap()
    sel_in = nc.dram_tensor("sel", [4, 32], BF16, kind="ExternalInput").ap()
    qkv_w = nc.dram_tensor("qkv_wt", [L, 128, NC16 * QKV_PC], BF16,
                           kind="ExternalInput").ap()
    f1_w = nc.dram_tensor("ffn1_wt", [L, 128, NC16 * FFN_PC], BF16,
                          kind="ExternalInput").ap()
    f2_w = nc.dram_tensor("ffn2_wt", [L, 128, 8 * HID], BF16,
                          kind="ExternalInput").ap()
    ow = nc.dram_tensor("out_wt", [128, NC16 * VPCP], BF16,
                        kind="ExternalInput").ap()
    ktall_d = nc.dram_tensor("ktall", [L, 128, PAIRS * KW], BF16,
                             kind="ExternalInput").ap()
    vtall_d = nc.dram_tensor("vtall", [L, 128, PAIRS * VW], BF16,
                             kind="ExternalInput").ap()
    out = nc.dram_tensor("out", [128, 64], F32, kind="ExternalOutput").ap()

    rsem = [nc.alloc_semaphore(f"rsem{p}") for p in range(2)]
    lsem = nc.alloc_semaphore("lsem")
    bsem = nc.alloc_semaphore("bsem")
    blsem = nc.alloc_semaphore("blsem")
    nwait = [0, 0]
    nround = [0]

    with tile.TileContext(nc) as tc:
        with (
            tc.tile_pool(name="sb", bufs=3) as sb,
            tc.tile_pool(name="wq_p", bufs=2) as wq_p,
            tc.tile_pool(name="f1_p", bufs=3) as f1_p,
            tc.tile_pool(name="f2_p", bufs=3) as f2_p,
            tc.tile_pool(name="kt_p", bufs=2) as kt_p,
            tc.tile_pool(name="vt_p", bufs=2) as vt_p,
            tc.tile_pool(name="ps", bufs=3, space="PSUM") as ps,
        ):
            # ---- cross-core entry barrier: align all 8 cores before any
            # real work so the measured span excludes PJRT dispatch skew ----
            if os.environ.get("KBARRIER", "1") == "1":
                tokr = sb.tile([128, 1], F32, bufs=1, name="tokr")
                nc.gpsimd.memset(tokr[:], 0.0)
                with tc.tile_critical():
                    rd = [None] + [(0, j) for j in range(1, 8)]
                    nc.gpsimd.remote_dma_broadcast(
                        tokr[:], tokr[:], remote_sem=bsem, local_sem=blsem,
                        rdests=rd)
                    nc.gpsimd.trigger_dma(count=None)
                    nc.gpsimd.wait_ge(bsem, 14)
                    nc.vector.wait_ge(bsem, 14)
                    nc.scalar.wait_ge(bsem, 14)
                    nc.sync.wait_ge(bsem, 14)
                    nc.tensor.wait_ge(bsem, 14)

            # ---- persistent small tiles ----
            XT = sb.tile([128, 32], F32, bufs=1, name="XT")
            gT = sb.tile([128, 16], F32, bufs=1, name="gT")
            bT = sb.tile([128, 16], F32, bufs=1, name="bT")
            ones = sb.tile([128, 1], F32, bufs=1, name="ones")
            ident = sb.tile([128, 128], BF16, bufs=1, name="ident")
            selm = sb.tile([4, 32], BF16, bufs=1, name="selm")
            nc.scalar.dma_start(XT[:], xpet[:])
            nc.scalar.dma_start(gT[:], lng[:])
            nc.scalar.dma_start(bT[:], lnb[:])
            nc.scalar.dma_start(ident[:], ident_in[:])
            nc.scalar.dma_start(selm[:], sel_in[:])
            nc.vector.memset(ones[:], 1.0)
            onesr = sb.tile([1, 128], F32, bufs=1, name="onesr")
            nc.vector.memset(onesr[:], 1.0)
            eps = sb.tile([1, 1], F32, bufs=1, name="eps")
            nc.vector.memset(eps[:], 1e-5)
            knew = [sb.tile([128, 4], BF16, bufs=1, name=f"knew{l}")
                    for l in range(L - 2)]
            vnew = [sb.tile([4, 128], BF16, bufs=1, name=f"vnew{l}")
                    for l in range(L - 2)]
            # collective buffers: parity double-buffered
            stg = [sb.tile([128, 32], F32, bufs=1, name=f"stg{p}")
                   for p in range(2)]
            agb = [sb.tile([128, 8, 32], F32, bufs=1, name=f"agb{p}")
                   for p in range(2)]

            def allred(src_ps, name):
                """Sum src_ps [128,32] (PSUM) across all 8 cores -> sbuf tile."""
                r = nround[0]
                nround[0] += 1
                p = r & 1
                if r >= 2:
                    with tc.tile_critical():
                        nc.vector.wait_ge(lsem, 112 * r)
                        nc.vector.tensor_copy(stg[p][:], src_ps)
                else:
                    nc.vector.tensor_copy(stg[p][:], src_ps)
                nc.vector.tensor_copy(agb[p][:, 0, :], stg[p][:])
                for j in range(1, 8):
                    rd = [None] * 8
                    rd[j] = (0, j)
                    nc.gpsimd.remote_dma_broadcast(
                        agb[p][:, j, :], stg[p][:],
                        remote_sem=rsem[p], local_sem=lsem, rdests=rd)
                nc.gpsimd.trigger_dma(count=None)
                nwait[p] += 14
                with tc.tile_critical():
                    nc.vector.wait_ge(rsem[p], nwait[p])
                    res = sb.tile([128, 32], F32, tag="ar_res",
                                  name=f"res_{name}")
                    nc.vector.reduce_sum(
                        res[:], agb[p][:, :, :].rearrange("p s f -> p f s"),
                        axis=AX.X)
                return res

            # ---- chunked streaming loads, all on the sync HWDGE queue ----
            def issue(pool, tag, name, src_ap, cols):
                t = pool.tile([128, cols], BF16, tag=tag, name=name)
                nc.sync.dma_start(t[:], src_ap)
                return t

            def issue_wq(l1):
                h = 8 * QKV_PC
                return [issue(wq_p, "wq", f"wq{l1}a", qkv_w[l1, :, 0:h], h),
                        issue(wq_p, "wq", f"wq{l1}b", qkv_w[l1, :, h:2 * h], h)]

            def issue_kt(l1):
                return [issue(kt_p, "kt", f"kt{l1}a",
                              ktall_d[l1, :, 0:2 * KW], 2 * KW),
                        issue(kt_p, "kt", f"kt{l1}b",
                              ktall_d[l1, :, 2 * KW:4 * KW], 2 * KW)]

            def issue_vt(l1):
                return [issue(vt_p, "vt", f"vt{l1}a",
                              vtall_d[l1, :, 0:2 * VW], 2 * VW),
                        issue(vt_p, "vt", f"vt{l1}b",
                              vtall_d[l1, :, 2 * VW:4 * VW], 2 * VW)]

            def issue_f1(l1):
                h = 8 * FFN_PC
                return [issue(f1_p, "f1", f"f1{l1}a", f1_w[l1, :, 0:h], h),
                        issue(f1_p, "f1", f"f1{l1}b", f1_w[l1, :, h:2 * h], h)]

            def issue_f2(l1):
                h = 4 * HID
                return [issue(f2_p, "f2", f"f2{l1}a", f2_w[l1, :, 0:h], h),
                        issue(f2_p, "f2", f"f2{l1}b", f2_w[l1, :, h:2 * h], h)]

            owt = {}

            def issue_ow(k, pool, tag):
                owt[k] = issue(pool, tag, f"ow{k}",
                               ow[:, VPCP * k:VPCP * (k + 1)], VPCP)

            # prologue: layer 0 chunks in consumption order
            cur = {
                "wq": issue_wq(0), "kt": issue_kt(0), "vt": issue_vt(0),
                "f1": issue_f1(0), "f2": issue_f2(0),
            }

            for l in range(L):
                nxt = {}
                # ---------- LN1 + QKV ----------
                xn, xnb = _ln(nc, sb, ps, XT, gT, bT, ones, onesr, eps, f"l{l}a")
                psq = ps.tile([128, 12], F32, tag="ps_big", name=f"psq{l}")
                for ch in range(NC16):
                    wt = cur["wq"][ch // 8]
                    base = QKV_PC * (ch % 8)
                    rhs = xnb[:, ch::16]
                    for j in range(6):
                        nc.tensor.matmul(
                            psq[:, 2 * j:2 * j + 2],
                            wt[:, base + 128 * j:base + 128 * (j + 1)],
                            rhs, start=(ch == 0 and j == 0),
                            stop=(ch == NC16 - 1 and j == 5),
                        )
                if l + 1 < L:
                    nxt["wq"] = issue_wq(l + 1)
                else:
                    issue_ow(0, wq_p, "wq")
                    issue_ow(1, wq_p, "wq")
                # extract qT, k_new, v_newT; psq col = 6*jh + 2*t + b
                qT = sb.tile([128, 4], BF16, tag="qT", name=f"qT{l}")
                for b in range(2):
                    nc.vector.tensor_copy(qT[:, 2 * b:2 * b + 2],
                                          psq[:, b:b + 7:6])
                if l < L - 2:
                    vnT = sb.tile([128, 4], BF16, tag="vnT", name=f"vnT{l}")
                    for b in range(2):
                        nc.vector.tensor_copy(knew[l][:, 2 * b:2 * b + 2],
                                              psq[:, 2 + b:2 + b + 7:6])
                        nc.vector.tensor_copy(vnT[:, 2 * b:2 * b + 2],
                                              psq[:, 4 + b:4 + b + 7:6])
                    ptr = ps.tile([4, 128], BF16, tag="ps_small",
                                  name=f"ptr{l}")
                    nc.tensor.transpose(ptr[:], vnT[:], ident[:])
                    nc.vector.tensor_copy(vnew[l][:], ptr[:])

                # ---------- attention ----------
                ktc, vtc = cur["kt"], cur["vt"]
                # inject new KV rows computed on-device
                for pi in range(PAIRS):
                    kt = ktc[pi // 2]
                    vt = vtc[pi // 2]
                    ko = (pi % 2) * KW
                    vo = (pi % 2) * VW
                    if l == 0:
                        nc.vector.tensor_copy(
                            kt[:, ko + 2048:ko + 2049], knew[0][:, pi:pi + 1])
                        nc.gpsimd.dma_start(
                            vt[0:1, vo + 2048:vo + 2176], vnew[0][pi:pi + 1, :])
                    elif l >= 2:
                        nc.vector.tensor_copy(
                            kt[:, ko + 1023:ko + 1024],
                            knew[l - 2][:, pi:pi + 1])
                        nc.gpsimd.dma_start(
                            vt[127:128, vo + 896:vo + 1024],
                            vnew[l - 2][pi:pi + 1, :])
                pss = ps.tile([128, PAIRS * 16], F32, tag="ps_big",
                              name=f"pss{l}")
                ps96 = ps.tile([1, 4], F32, tag="ps_small", name=f"ps96{l}")
                for pi in range(PAIRS):
                    kt = ktc[pi // 2]
                    ko = (pi % 2) * KW
                    for c in range(16):
                        nc.tensor.matmul(
                            pss[:, 16 * pi + c:16 * pi + c + 1],
                            kt[:, ko + 128 * c:ko + 128 * (c + 1)],
                            qT[:, pi:pi + 1], start=True, stop=True,
                        )
                    nc.tensor.matmul(
                        ps96[0:1, pi:pi + 1],
                        kt[:, ko + 2048:ko + 2049],
                        qT[:, pi:pi + 1], start=True, stop=True,
                    )
                if l + 1 < L:
                    nxt["kt"] = issue_kt(l + 1)
                else:
                    issue_ow(2, kt_p, "kt")
                    issue_ow(3, kt_p, "kt")
                prob = sb.tile([128, PAIRS * 16], F32, tag="prob",
                               name=f"prob{l}")
                nc.scalar.activation(prob[:], pss[:], AF.Exp, scale=INVSQ)
                p96 = sb.tile([1, 4], F32, tag="p96", name=f"p96{l}")
                nc.scalar.activation(p96[:], ps96[:], AF.Exp, scale=INVSQ)
                ssum = sb.tile([1, 4], F32, tag="ssum", name=f"ssum{l}")
                pssum = ps.tile([1, PAIRS * 16], F32, tag="ps_small",
                                name=f"pssum{l}")
                nc.tensor.matmul(pssum[:], ones[:], prob[:], start=True,
                                 stop=True)
                sumsb = sb.tile([1, PAIRS * 16], F32, tag="sumsb",
                                name=f"sumsb{l}")
                nc.vector.tensor_copy(sumsb[:], pssum[:])
                nc.vector.reduce_sum(
                    ssum[:],
                    sumsb[0:1, :].rearrange("p (q c) -> p q c", q=PAIRS),
                    axis=AX.X,
                )
                nc.vector.tensor_add(ssum[:], ssum[:], p96[:])
                inv = sb.tile([1, 4], F32, tag="inv", name=f"inv{l}")
                nc.vector.reciprocal(inv[:], ssum[:])
                p96b = sb.tile([1, 4], BF16, tag="p96b", name=f"p96b{l}")
                nc.vector.tensor_mul(p96b[:], p96[:], inv[:])
                invb = ps.tile([128, 4], F32, tag="ps_small",
                               name=f"invb{l}")
                nc.tensor.matmul(invb[:], onesr[0:1, :], inv[0:1, :],
                                 start=True, stop=True)
                prob_b = sb.tile([128, PAIRS * 16], BF16, tag="prob_b",
                                 name=f"prob_b{l}")
                nc.vector.tensor_mul(
                    prob_b[:, :].rearrange("p (q c) -> p q c", q=PAIRS),
                    prob[:, :].rearrange("p (q c) -> p q c", q=PAIRS),
                    invb[:, :].unsqueeze(2).broadcast_to([128, PAIRS, 16]),
                )
                pso = ps.tile([128, 4], F32, tag="ps_big", name=f"pso{l}")
                for pi in range(PAIRS):
                    vt = vtc[pi // 2]
                    vo = (pi % 2) * VW
                    for c in range(16):
                        nc.tensor.matmul(
                            pso[:, pi:pi + 1],
                            vt[:, vo + 128 * c:vo + 128 * (c + 1)],
                            prob_b[:, 16 * pi + c:16 * pi + c + 1],
                            start=(c == 0), stop=False,
                        )
                    nc.tensor.matmul(
                        pso[:, pi:pi + 1],
                        vt[0:1, vo + 2048:vo + 2176],
                        p96b[0:1, pi:pi + 1],
                        start=False, stop=True,
                    )
                if l + 1 < L:
                    nxt["vt"] = issue_vt(l + 1)
                else:
                    issue_ow(4, vt_p, "vt")
                    issue_ow(5, vt_p, "vt")
                o_sb = sb.tile([128, 4], BF16, tag="o_sb", name=f"o{l}")
                nc.vector.tensor_copy(o_sb[:], pso[:])

                # place own heads into XT columns: stage = o @ E
                potr = ps.tile([4, 128], BF16, tag="ps_small", name=f"potr{l}")
                nc.tensor.transpose(potr[:], o_sb[:], ident[:])
                oT = sb.tile([4, 128], BF16, tag="oT", name=f"oT{l}")
                nc.vector.tensor_copy(oT[:], potr[:])
                pstg = ps.tile([128, 32], F32, tag="ps_big", name=f"pstg{l}")
                nc.tensor.matmul(pstg[:], oT[:], selm[:], start=True,
                                 stop=True)
                delta = allred(pstg[:, :], f"attn{l}")
                nc.vector.tensor_add(XT[:], XT[:], delta[:])

                # ---------- LN2 + FFN ----------
                xn2, xnb2 = _ln(nc, sb, ps, XT, gT, bT, ones, onesr, eps, f"l{l}b")
                psh = ps.tile([128, 16], F32, tag="ps_big", name=f"psh{l}")
                for ch in range(NC16):
                    wt1 = cur["f1"][ch // 8]
                    base = FFN_PC * (ch % 8)
                    rhs = xnb2[:, ch::16]
                    for j in range(8):
                        nc.tensor.matmul(
                            psh[:, 2 * j:2 * j + 2],
                            wt1[:, base + 128 * j:base + 128 * (j + 1)],
                            rhs, start=(ch == 0 and j == 0),
                            stop=(ch == NC16 - 1 and j == 7),
                        )
                if l + 1 < L:
                    nxt["f1"] = issue_f1(l + 1)
                else:
                    issue_ow(6, f1_p, "f1")
                    issue_ow(7, f1_p, "f1")
                    issue_ow(8, f1_p, "f1")
                hT = sb.tile([128, 16], BF16, tag="hT", name=f"hT{l}")
                nc.scalar.activation(hT[:], psh[:], AF.Gelu)
                psf = ps.tile([128, 32], F32, tag="ps_big", name=f"psf{l}")
                for ck in range(8):
                    wt2 = cur["f2"][ck // 4]
                    base = HID * (ck % 4)
                    rhs = hT[:, 2 * ck:2 * ck + 2]
                    for m in range(16):
                        nc.tensor.matmul(
                            psf[:, 2 * m:2 * m + 2],
                            wt2[:, base + 128 * m:base + 128 * (m + 1)],
                            rhs, start=(ck == 0 and m == 0),
                            stop=(ck == 7 and m == 15),
                        )
                if l + 1 < L:
                    nxt["f2"] = issue_f2(l + 1)
                else:
                    issue_ow(9, f2_p, "f2")
                    issue_ow(10, f2_p, "f2")
                    issue_ow(11, f2_p, "f2")
                ard = allred(psf[:, :], f"ffn{l}")
                nc.vector.tensor_add(
                    XT[:, :].rearrange("p (b c) -> p b c", b=2),
                    XT[:, :].rearrange("p (b c) -> p b c", b=2),
                    ard[:, :].rearrange("p (m b) -> p b m", b=2),
                )
                cur = nxt

            # ---------- final LN + vocab head ----------
            xn3, xnb3 = _ln(nc, sb, ps, XT, gT, bT, ones, onesr, eps, "fin")
            psl = ps.tile([128, 64], F32, tag="ps_big", name="psl")
            for k in range(NC16):
                wto = owt[k]
                rhs = xnb3[:, k::16]
                for m in range(32):
                    nc.tensor.matmul(
                        psl[:, 2 * m:2 * m + 2],
                        wto[:, 128 * m:128 * (m + 1)],
                        rhs, start=(k == 0 and m == 0),
                        stop=(k == NC16 - 1 and m == 31),
                    )
                if k == 1:
                    issue_ow(12, wq_p, "wq")
                    issue_ow(13, wq_p, "wq")
                elif k == 3:
                    issue_ow(14, kt_p, "kt")
                    issue_ow(15, kt_p, "kt")
            logT = sb.tile([128, 64], F32, bufs=1, name="logT")
            nc.vector.tensor_copy(logT[:], psl[:])
            E = sb.tile([128, 64], F32, bufs=1, name="E")
            nc.scalar.activation(E[:], logT[:], AF.Exp)
            nc.sync.dma_start(out[:], E[:])

    nc.compile()
    return nc


def _get_nc():
    global _cached
    if _cached is None:
        _cached = _build()
    return _cached


def _pos_encoding(pos):
    half = np.arange(HID // 2, dtype=np.float32)
    div = np.exp((-math.log(10000.0) * (2.0 * half) / HID).astype(np.float32))
    ang = np.float32(pos) * div
    pe = np.zeros((HID,), dtype=np.float32)
    pe[0::2] = np.sin(ang)
    pe[1::2] = np.cos(ang)
    return pe


def kernel(x, qkv_w, ffn1_w, ffn2_w, out_w, ln_g, ln_b,
           k_heap, v_heap, k_off, v_off, current_pos):
    import ml_dtypes
    bf16 = ml_dtypes.bfloat16

    x = np.asarray(x, dtype=np.float32)
    qkv_w = np.asarray(qkv_w, dtype=np.float32)
    ffn1_w = np.asarray(ffn1_w, dtype=np.float32)
    ffn2_w = np.asarray(ffn2_w, dtype=np.float32)
    out_w = np.asarray(out_w, dtype=np.float32)
    ln_g = np.asarray(ln_g, dtype=np.float32)
    ln_b = np.asarray(ln_b, dtype=np.float32)
    k_heap = np.asarray(k_heap, dtype=np.float32)
    v_heap = np.asarray(v_heap, dtype=np.float32)
    k_off = np.asarray(k_off, dtype=np.float32)
    v_off = np.asarray(v_off, dtype=np.float32)
    pos = int(np.asarray(current_pos))

    xpe = x.reshape(B, HID) + _pos_encoding(pos)[None, :]
    xpet = np.ascontiguousarray(
        xpe.reshape(B, NC16, 128).transpose(2, 0, 1).reshape(128, B * NC16))
    lng_t = np.ascontiguousarray(ln_g.reshape(NC16, 128).T)
    lnb_t = np.ascontiguousarray(ln_b.reshape(NC16, 128).T)
    ident = np.eye(128, dtype=bf16)

    in_maps = []
    for c in range(NCORES):
        # weights: [L, 128, chunks*out] with wt[l, p, ch*O + o] = W[l, o_row, ch*128+p]
        qs = qkv_w[:, QKV_PC * c:QKV_PC * (c + 1), :]        # [L, 768, 2048]
        qs = qs.reshape(L, QKV_PC, NC16, 128).transpose(0, 3, 2, 1)
        qs = np.ascontiguousarray(qs.reshape(L, 128, NC16 * QKV_PC).astype(bf16))
        f1 = ffn1_w[:, FFN_PC * c:FFN_PC * (c + 1), :]       # [L, 1024, 2048]
        f1 = f1.reshape(L, FFN_PC, NC16, 128).transpose(0, 3, 2, 1)
        f1 = np.ascontiguousarray(f1.reshape(L, 128, NC16 * FFN_PC).astype(bf16))
        f2 = ffn2_w[:, :, FFN_PC * c:FFN_PC * (c + 1)]       # [L, 2048, 1024]
        f2 = f2.transpose(0, 2, 1).reshape(L, 8, 128, HID).transpose(0, 2, 1, 3)
        f2 = np.ascontiguousarray(f2.reshape(L, 128, 8 * HID).astype(bf16))
        owt = np.zeros((128, NC16 * VPCP), dtype=bf16)
        ow_c = out_w[VPC * c:VPC * (c + 1), :]               # [4000, 2048]
        ow_r = np.zeros((VPCP, HID), dtype=np.float32)
        ow_r[:VPC] = ow_c
        # owt[p, ch*VPCP + o] = ow_r[o, ch*128+p]
        owt[:] = ow_r.reshape(VPCP, NC16, 128).transpose(2, 1, 0).reshape(
            128, NC16 * VPCP).astype(bf16)

        h0, h1 = HPC * c, HPC * (c + 1)
        kh = k_heap[:, h0:h1].reshape(PAIRS, -1, 128)        # [4, P, 128]
        vh = v_heap[:, h0:h1].reshape(PAIRS, -1, 128)
        ko = k_off[:, :, h0:h1].reshape(L - 1, PAIRS, OFF, 128)
        vo = v_off[:, :, h0:h1].reshape(L - 1, PAIRS, OFF, 128)
        ktall = np.zeros((L, 128, PAIRS * KW), dtype=bf16)
        vtall = np.zeros((L, 128, PAIRS * VW), dtype=bf16)
        for l in range(L):
            if l == 0:
                kc = kh[:, 0:KW]                             # [4, 2049, 128]
                vc = vh[:, 0:KW]
            else:
                rs = (l - 1) * R
                kc = np.concatenate([kh[:, rs:rs + R], ko[l - 1]], axis=1)
                vc = np.concatenate([vh[:, rs:rs + R], vo[l - 1]], axis=1)
            ktall[l] = kc.transpose(2, 0, 1).reshape(128, PAIRS * KW).astype(bf16)
            vp = np.zeros((PAIRS, VW, 128), dtype=np.float32)
            vp[:, :KW] = vc
            # vt[p, pi*VW + chd] with chd = ch*128+d, row = ch*128+p
            vtall[l] = vp.reshape(PAIRS, KCH, 128, 128).transpose(
                2, 0, 1, 3).reshape(128, PAIRS * VW).astype(bf16)

        # selection matrix: E[pi=2b+j, b*16 + 2c + j] = 1
        sel = np.zeros((4, 32), dtype=bf16)
        for b in range(2):
            for j in range(2):
                sel[2 * b + j, b * 16 + 2 * c + j] = 1
        in_maps.append({
            "xpet": xpet, "lng_t": lng_t, "lnb_t": lnb_t, "ident": ident,
            "sel": sel, "qkv_wt": qs, "ffn1_wt": f1, "ffn2_wt": f2,
            "out_wt": owt, "ktall": np.ascontiguousarray(ktall),
            "vtall": np.ascontiguousarray(vtall),
        })

    nc = _get_nc()
    try:
        res = run_bass_kernel_spmd(nc, in_maps, core_ids=list(range(NCORES)))
    except ModuleNotFoundError:
        os.environ["BASS_NEVER_TRACE"] = "1"
        res = run_bass_kernel_spmd(nc, in_maps, core_ids=list(range(NCORES)))
    global LAST_RESULT
    LAST_RESULT = res

    expv = np.zeros((B, V), dtype=np.float32)
    for c in range(NCORES):
        o = res.results[c]["out"].reshape(128, 32, 2)
        for b in range(B):
            expv[b, VPC * c:VPC * (c + 1)] = \
                o[:, :, b].T.reshape(VPCP)[:VPC]
    probs = expv / expv.sum(axis=1, keepdims=True)
    return probs.reshape(B, 1, V).astype(np.float32)
